# revision 46
# baseline (speedup 1.0000x reference)
"""AttentionBlock (GroupNorm + single-head NxN attention + residual) on 8 TRN2 cores.

Data-parallel: batch dim (B=8) sharded 1 image per NeuronCore. Each core runs
the full block for its image:

  x (C=256, N=4096) -> GroupNorm(8 groups) -> q,k = Wq,Wk @ xn (fp8)
  u = (W_out @ W_v) @ xn (fp8, output projection folded into V)
  s = k^T q (fp8 DoubleRow matmul, contraction C=256 in one pass)
  e = exp(s/16 - 2) (scalar engine, fp8 out; the -2 shift cancels in softmax)
  attn_u = u @ e, den = ones @ e (fp8 DoubleRow)
  out = attn_u * (1/den) + b_out' + x

All heavy matmuls run as fp8e4m3 with MatmulPerfMode.DoubleRow (K=256 per
matmul). The softmax is unnormalized; 1/den commutes through the (folded)
output projection and is applied once at the end. fp8 noise only touches the
attention branch, which is small versus the fp32 residual, keeping max-rel
error ~1e-3.
"""

import sys

if "/opt/trn_rl_repo" not in sys.path:
    sys.path.insert(0, "/opt/trn_rl_repo")

import numpy as np
import ml_dtypes

import concourse.bass as bass
import concourse.bacc as bacc
import concourse.tile as tile
import concourse.mybir as mybir
from concourse import bass_utils

# Problem dims (hardcoded per spec)
B, C, HH, WW = 8, 256, 64, 64
N = HH * WW            # 4096
G = 8                  # groupnorm groups
GSZ = C // G           # 32 channels/group
EPS = 1e-5
P = 128                # SBUF partitions
CT = C // P            # 2 channel tiles (also the DoubleRow K-tile count)
NCH = 512              # query-chunk width (free dim per matmul)
NNCH = N // NCH        # 8
MT = N // P            # 32 key tiles of 128
JT = MT // 2           # 16 key supertiles of 256 (DoubleRow)
SCALE = 1.0 / np.sqrt(C)
SHIFT = 4.0            # exp(s*SCALE - SHIFT); cancels in softmax, keeps e well
                       # inside fp8e4m3 range (max observed s*SCALE is ~8 ->
                       # e^4 = 55 << 240; overflow headroom up to s*SCALE=9.4)
INV_CNT = 1.0 / (GSZ * N)

F32 = mybir.dt.float32
F32R = mybir.dt.float32r
F8 = mybir.dt.float8e4
BF16 = mybir.dt.bfloat16
DR = mybir.MatmulPerfMode.DoubleRow
NP_F8 = ml_dtypes.float8_e4m3
NP_BF16 = ml_dtypes.bfloat16


def _emit(tc, d, out_d):
    from contextlib import ExitStack

    nc = tc.nc
    AF = mybir.ActivationFunctionType
    OP = mybir.AluOpType
    AX = mybir.AxisListType.X
    ts, ds = bass.ts, bass.ds

    with ExitStack() as ctx:
        const = ctx.enter_context(tc.tile_pool(name="const", bufs=1))
        big = ctx.enter_context(tc.tile_pool(name="big", bufs=1))
        work = ctx.enter_context(tc.tile_pool(name="work", bufs=4))
        small = ctx.enter_context(tc.tile_pool(name="small", bufs=4))
        outp = ctx.enter_context(tc.tile_pool(name="outp", bufs=3))

        # ---------------- load x (bf16 copy first; fp32 in background) ------
        # xb (bf16) feeds GroupNorm stats + xn and lands fast; the fp32 x is
        # only needed for the residual add during finalize (~40us later), so
        # its DMA is issued after everything else on the rings.
        NC4 = 4                      # chunks per channel-tile
        CW = N // NC4                # 1024 columns per chunk
        x_d = d["x"]
        xb_d = d["xb"]
        x_sb = big.tile([P, CT, N], F32, name="x_sb")
        xb_sb = big.tile([P, CT, N], BF16, name="xb_sb")
        dma_engs = (nc.sync, nc.gpsimd, nc.scalar)
        for t in range(CT):
            for c in range(NC4):
                eng = dma_engs[(t * NC4 + c) % len(dma_engs)]
                eng.dma_start(out=xb_sb[:, t, ds(c * CW, CW)],
                              in_=xb_d[ts(t, P), ds(c * CW, CW)])

        fm_sb = const.tile([P, CT, G], F32, name="fm_sb")
        bm_sb = const.tile([G, CT, P], F32, name="bm_sb")
        for t in range(CT):
            nc.sync.dma_start(out=fm_sb[:, t, :], in_=d["fmask"][t])
            nc.sync.dma_start(out=bm_sb[:, t, :], in_=d["bmask"][t])

        # bias5 rows: 0=b_q 1=b_k 2=b_o 3=gn_w 4=gn_b; SBUF [P, 5, CT]
        b5_sb = const.tile([P, 5, CT], F32, name="b5_sb")
        nc.sync.dma_start(out=b5_sb, in_=d["bias5"])
        bq_sb = b5_sb[:, 0, :]
        bk_sb = b5_sb[:, 1, :]
        bo_sb = b5_sb[:, 2, :]
        gw_sb = b5_sb[:, 3, :]
        gb_sb = b5_sb[:, 4, :]

        wq_sb = const.tile([P, CT, C], F8, name="wq_sb")
        wk_sb = const.tile([P, CT, C], F8, name="wk_sb")
        wov_sb = const.tile([P, CT, C], F8, name="wov_sb")
        nc.sync.dma_start(out=wq_sb, in_=d["wq_p"])
        nc.sync.dma_start(out=wk_sb, in_=d["wk_p"])
        nc.sync.dma_start(out=wov_sb, in_=d["wov_p"])

        ones_sb = const.tile([P, CT, P], F8, name="ones_sb")
        nc.vector.memset(ones_sb, 1.0)
        zero_bf = const.tile([P, 1], BF16, name="zero_bf")
        nc.vector.memset(zero_bf, 0.0)
        nshift_sb = const.tile([P, 1], F32, name="nshift_sb")
        nc.vector.memset(nshift_sb, -SHIFT)
        eps_sb = const.tile([G, 1], F32, name="eps_sb")
        nc.vector.memset(eps_sb, EPS)

        # ------- GroupNorm stats: bn_stats for t0 (vector) + sums/squares
        # for t1 (vector+scalar). fmask is pre-scaled host-side so the group
        # matmul directly yields (mean_g, E[x^2]_g).
        NC8 = 8                      # bn_stats chunks (<=512 wide) per ct
        SW = N // NC8                # 512
        st6 = small.tile([P, NC8, 6], F32, name="st6")
        cstat = small.tile([P, 2], F32, name="cstat")   # t0 (mean, var)
        mex = small.tile([P, 2], F32, name="mex")       # t0 (mean, E[x^2])
        stat1 = small.tile([P, NC4, 2], F32, name="stat1")  # t1 (sum, sqsum)
        sqscr = small.tile([P, CW], F32, name="sqscr", bufs=2)
        zero_f = const.tile([P, 1], F32, name="zero_f")
        nc.vector.memset(zero_f, 0.0)
        ab = small.tile([P, CT, 2], F32, name="ab")  # per-channel scale, bias
        with tc.tile_pool(name="psI", bufs=1, space="PSUM") as psI:
            for c in range(NC8):
                nc.vector.bn_stats(out=st6[:, c, :],
                                   in_=xb_sb[:, 0, ds(c * SW, SW)])
                if c % 2 == 0:
                    # PE warm-up: bf16 matmul on the freshly-landed chunk
                    warm = psI.tile([1, NCH], F32, tag="warm", name="warm")
                    nc.tensor.matmul(warm, lhsT=zero_bf,
                                     rhs=xb_sb[:, 0, ds(c * SW, NCH)],
                                     start=True, stop=True)
            for c in range(NC4):
                csl = ds(c * CW, CW)
                nc.scalar.activation(out=sqscr, in_=xb_sb[:, 1, csl],
                                     func=AF.Square, bias=zero_f,
                                     accum_out=stat1[:, c, 1:2])
                nc.vector.reduce_sum(out=stat1[:, c, 0:1],
                                     in_=xb_sb[:, 1, csl], axis=AX)
            nc.vector.bn_aggr(out=cstat, in_=st6)
            # mex = (mean, E[x^2]) for the group aggregation
            nc.vector.tensor_copy(out=mex[:, 0:1], in_=cstat[:, 0:1])
            nc.vector.tensor_mul(out=mex[:, 1:2], in0=cstat[:, 0:1],
                                 in1=cstat[:, 0:1])
            nc.vector.tensor_add(out=mex[:, 1:2], in0=mex[:, 1:2],
                                 in1=cstat[:, 1:2])

            gps = psI.tile([G, 2], F32, tag="gps", name="gps")
            nc.tensor.matmul(gps, lhsT=fm_sb[:, 0, :], rhs=mex,
                             start=True, stop=False)
            for c in range(NC4):
                nc.tensor.matmul(gps, lhsT=fm_sb[:, 1, :], rhs=stat1[:, c, :],
                                 start=False, stop=(c == NC4 - 1))
            grp = small.tile([G, 2], F32, name="grp")    # [mean, rstd]
            gtmp = small.tile([G, 3], F32, name="gtmp")
            nc.vector.tensor_copy(out=grp[:, 0:1], in_=gps[:, 0:1])
            nc.vector.tensor_mul(out=gtmp[:, 1:2], in0=grp[:, 0:1], in1=grp[:, 0:1])
            nc.vector.tensor_sub(out=gtmp[:, 2:3], in0=gps[:, 1:2], in1=gtmp[:, 1:2])
            nc.scalar.activation(out=gtmp[:, 2:3], in_=gtmp[:, 2:3], func=AF.Sqrt,
                                 bias=eps_sb)
            nc.vector.reciprocal(out=grp[:, 1:2], in_=gtmp[:, 2:3])
            # preload the Exp act table while PE/vector finish the ab chain
            nc.scalar.activation(out=gtmp[:, 0:1], in_=gtmp[:, 1:2],
                                 func=AF.Exp, bias=eps_sb)

            cps = psI.tile([P, CT, 2], F32, tag="cps", name="cps")
            for t in range(CT):
                nc.tensor.matmul(cps[:, t, :], lhsT=bm_sb[:, t, :], rhs=grp,
                                 start=True, stop=True)
            nc.vector.tensor_mul(out=ab[:, :, 0], in0=cps[:, :, 1],
                                 in1=gw_sb)
            nc.vector.tensor_mul(out=ab[:, :, 1], in0=cps[:, :, 0],
                                 in1=ab[:, :, 0])
            nc.vector.tensor_sub(out=ab[:, :, 1], in0=gb_sb,
                                 in1=ab[:, :, 1])

        # ---------------- xn = a*x + b in fp8 (vector) ----------------------
        xn_sb = big.tile([P, CT, N], F8, name="xn_sb")
        for t in (1, 0):
            nc.vector.tensor_scalar(out=xn_sb[:, t, :], in0=xb_sb[:, t, :],
                                    scalar1=ab[:, t, 0:1], scalar2=ab[:, t, 1:2],
                                    op0=OP.mult, op1=OP.add)

        # fp32 x for the residual: issued now, needed from the first finalize
        for t in range(CT):
            for c in range(NC4):
                eng = (nc.sync, nc.gpsimd)[(t * NC4 + c) % 2]
                eng.dma_start(out=x_sb[:, t, ds(c * CW, CW)],
                              in_=x_d[ts(t, P), ds(c * CW, CW)])

        q_sb = big.tile([P, CT, N], F8, name="q_sb")   # (c, n)
        k_sb = big.tile([P, CT, N], F8, name="k_sb")
        u_sb = big.tile([P, JT, CT, C], F8, name="u_sb")  # uT packed (m, c)

        def emit_q(pool, nch):
            """q chunk pair (both tq) for one nch; copies on vector."""
            nsl = ds(nch * NCH, NCH)
            pr = pool.tile([P, CT, NCH], F32, tag="s", name="psq")
            for tq in range(CT):
                nc.tensor.matmul(pr[:, tq, :], lhsT=wq_sb[:, :, ts(tq, P)],
                                 rhs=xn_sb[:, :, nsl],
                                 start=True, stop=True, perf_mode=DR)
            for tq in range(CT):
                nc.vector.tensor_scalar_add(out=q_sb[:, tq, nsl],
                                            in0=pr[:, tq, :],
                                            scalar1=bq_sb[:, tq:tq + 1])

        def emit_k(pool, c, eng_idx):
            """k chunk pair (both tq) for nch_k c; copies on scalar/vector."""
            nsl = ds(c * NCH, NCH)
            pr = pool.tile([P, CT, NCH], F32, tag="s", name="psk")
            for tq in range(CT):
                nc.tensor.matmul(pr[:, tq, :], lhsT=wk_sb[:, :, ts(tq, P)],
                                 rhs=xn_sb[:, :, nsl],
                                 start=True, stop=True, perf_mode=DR)
            for tq in range(CT):
                if eng_idx == 0:
                    nc.scalar.activation(out=k_sb[:, tq, nsl], in_=pr[:, tq, :],
                                         func=AF.Identity,
                                         bias=bk_sb[:, tq:tq + 1])
                else:
                    nc.vector.tensor_scalar_add(out=k_sb[:, tq, nsl],
                                                in0=pr[:, tq, :],
                                                scalar1=bk_sb[:, tq:tq + 1])

        def emit_u(pool, g, eng_idx):
            """u supertiles 2g, 2g+1 (mt 4g..4g+3); one whole-pair copy."""
            pr = pool.tile([P, CT, NCH], F32, tag="s", name="psu")
            for h in range(4):
                mt = 4 * g + h
                nc.tensor.matmul(pr[:, h // 2, ds((h % 2) * C, C)],
                                 lhsT=xn_sb[:, :, ts(mt, P)],
                                 rhs=wov_sb, start=True, stop=True, perf_mode=DR)
            dst = u_sb[:, 2 * g:2 * g + 2, :, :]
            if eng_idx == 0:
                nc.scalar.copy(out=dst, in_=pr)
            else:
                nc.vector.tensor_copy(out=dst, in_=pr)

        # -- pre-stage q0,q1, k c0-3, u g0-5; the rest rides the stream -------
        with tc.tile_pool(name="psQ", bufs=4, space="PSUM") as psQ:
            emit_q(psQ, 0)
            emit_k(psQ, 0, 0)
            emit_u(psQ, 0, 0)
            emit_q(psQ, 1)
            for c in range(1, 4):
                emit_k(psQ, c, 0)
            for g in range(1, 6):
                emit_u(psQ, g, 1 if g % 2 else 0)

        # ---------------- main PSUM pools (after psQ released) ---------------
        # 4 banks scores pairs + 2 attn + 1 den + 1 insertion = 8
        psS = ctx.enter_context(tc.tile_pool(name="psS", bufs=2, space="PSUM"))
        psA = ctx.enter_context(tc.tile_pool(name="psA", bufs=1, space="PSUM"))
        psD = ctx.enter_context(tc.tile_pool(name="psD", bufs=1, space="PSUM"))
        psU = ctx.enter_context(tc.tile_pool(name="psU", bufs=1, space="PSUM"))

        def ins_half(kind, a, b=0):
            """Single-bank in-stream production: one k/q chunk half or one u
            supertile, with its copy on vector."""
            pu = psU.tile([P, NCH], F32, tag="u1", name="pu")
            if kind == "u":
                for i in range(2):
                    nc.tensor.matmul(pu[:, ds(i * C, C)],
                                     lhsT=xn_sb[:, :, ts(2 * a + i, P)],
                                     rhs=wov_sb, start=True, stop=True,
                                     perf_mode=DR)
                nc.vector.tensor_copy(out=u_sb[:, a, :, :], in_=pu)
                return
            w_sb, b_sb, o_sb = ((wq_sb, bq_sb, q_sb) if kind == "q"
                                else (wk_sb, bk_sb, k_sb))
            nsl = ds(a * NCH, NCH)
            nc.tensor.matmul(pu, lhsT=w_sb[:, :, ts(b, P)],
                             rhs=xn_sb[:, :, nsl],
                             start=True, stop=True, perf_mode=DR)
            nc.vector.tensor_scalar_add(out=o_sb[:, b, nsl], in0=pu,
                                        scalar1=b_sb[:, b:b + 1])

        # deferred work for the nch0 sweep, deadline-ordered (k c needed at
        # pair j=2c; u supertile j' needed at attn j'); j0/j1 are reserved
        # for the ramp-up scores on psU
        INS0 = {2: ("k", 4, 0), 3: ("k", 4, 1), 4: ("k", 5, 0), 5: ("k", 5, 1),
                6: ("k", 6, 0), 7: ("k", 6, 1), 8: ("u", 12), 9: ("u", 13),
                10: ("k", 7, 0), 11: ("k", 7, 1), 12: ("u", 14), 13: ("u", 15)}

        # ---------------- attention + fused output projection ----------------
        # Software pipeline over all (nch, j): scores for step idx+1 are
        # emitted before attn/den of step idx so the PE keeps the scalar
        # engine's exp stream fed.
        steps = [(nch, j) for nch in range(NNCH) for j in range(JT)]

        def emit_scores(nch, j):
            nsl = ds(nch * NCH, NCH)
            pr = psS.tile([P, CT, NCH], F32, tag="s", name="pss")
            for i in range(2):
                mt = 2 * j + i
                nc.tensor.matmul(pr[:, i, :], lhsT=k_sb[:, :, ts(mt, P)],
                                 rhs=q_sb[:, :, nsl],
                                 start=True, stop=True, perf_mode=DR)
            return pr

        attn = None
        den = None
        pr_cur = None
        for idx, (nch, j) in enumerate(steps):
            nsl = ds(nch * NCH, NCH)
            if j == 0:
                attn = psA.tile([P, CT, NCH], F32, tag="attn", name="attn")
                den = psD.tile([P, NCH], F32, tag="den", name="den")
            e = work.tile([P, CT, NCH], F8, tag="e", name="e", bufs=4)
            if idx < 2:
                # ramp-up: single-bank scores on psU + single-tile exps, so
                # the exp stream starts before the prestage copies drain out
                # of the psQ banks that psS is about to reuse
                for i in range(2):
                    pu = psU.tile([P, NCH], F32, tag="u1", name="pss1")
                    nc.tensor.matmul(pu, lhsT=k_sb[:, :, ts(2 * j + i, P)],
                                     rhs=q_sb[:, :, nsl],
                                     start=True, stop=True, perf_mode=DR)
                    nc.scalar.activation(out=e[:, i, :], in_=pu, func=AF.Exp,
                                         bias=nshift_sb, scale=SCALE)
            else:
                nc.scalar.activation(out=e, in_=pr_cur, func=AF.Exp,
                                     bias=nshift_sb, scale=SCALE)
            if 2 <= idx + 1 < len(steps):
                pr_cur = emit_scores(*steps[idx + 1])
            if nch == 0:
                if j in INS0:
                    ins_half(*INS0[j])
            elif nch + 1 < NNCH and j in (9, 11):
                ins_half("q", nch + 1, 0 if j == 9 else 1)
            for co in range(CT):
                nc.tensor.matmul(attn[:, co, :],
                                 lhsT=u_sb[:, j, :, ts(co, P)],
                                 rhs=e,
                                 start=(j == 0), stop=(j == JT - 1),
                                 perf_mode=DR)
            nc.tensor.matmul(den, lhsT=ones_sb, rhs=e,
                             start=(j == 0), stop=(j == JT - 1),
                             perf_mode=DR)
            if j != JT - 1:
                continue

            # -------- finalize this nch --------
            # pull attn out of PSUM first so the next nch's attn matmuls can
            # restart the accumulation group without waiting on the chain
            att_sb = outp.tile([P, CT, NCH], F32, tag="att_sb", name="att_sb",
                               bufs=2)
            # two bank-sized copies so each attn PSUM bank frees independently;
            # for the last nch the scalar engine is done with exps, so it
            # takes them and shortens the serial tail on vector
            for co in range(CT):
                if nch == NNCH - 1:
                    nc.scalar.copy(out=att_sb[:, co, :], in_=attn[:, co, :])
                else:
                    nc.vector.tensor_copy(out=att_sb[:, co, :],
                                          in_=attn[:, co, :])
            rden = outp.tile([P, NCH], F32, tag="rden", name="rden", bufs=2)
            rscr = outp.tile([P, NCH], F32, tag="rscr", name="rscr", bufs=2)
            nc.vector.reciprocal_approx_accurate(out=rden, in_=den, scratch=rscr)
            for co in range(CT):
                f = outp.tile([P, NCH], F32, tag="fout", name="f", bufs=3)
                nc.vector.tensor_tensor(out=f, in0=att_sb[:, co, :], in1=rden,
                                        op=OP.mult)
                nc.vector.scalar_tensor_tensor(out=f, in0=f,
                                               scalar=bo_sb[:, co:co + 1],
                                               in1=x_sb[:, co, nsl],
                                               op0=OP.add, op1=OP.add)
                nc.sync.dma_start(out=out_d[ts(co, P), nsl], in_=f)


def build_program():
    nc = bacc.Bacc("TRN2", target_bir_lowering=False, debug=False, num_devices=B)
    d = {}

    def din(name, shape, dt_=F32):
        d[name] = nc.dram_tensor(name, list(shape), dt_, kind="ExternalInput").ap()

    din("x", (C, N))
    din("xb", (C, N), BF16)
    din("wq_p", (P, CT, C), F8)
    din("wk_p", (P, CT, C), F8)
    din("wov_p", (P, CT, C), F8)
    din("bias5", (P, 5, CT))
    din("fmask", (CT, P, G))
    din("bmask", (CT, G, P))
    out_d = nc.dram_tensor("out", [C, N], F32, kind="ExternalOutput").ap()

    with tile.TileContext(nc) as tc:
        _emit(tc, d, out_d)
    nc.compile()
    return nc


_PROG = None


def _get_program():
    global _PROG
    if _PROG is None:
        _PROG = build_program()
    return _PROG


def _pack_w(w):
    """[c_out, c_in] fp32 -> [p, r, c_out] fp8 with c_in = r*128 + p."""
    wt = np.ascontiguousarray(w.T)                   # [c_in, c_out]
    return np.ascontiguousarray(
        wt.reshape(CT, P, C).transpose(1, 0, 2)).astype(NP_F8)


def make_in_maps(inputs):
    x = np.ascontiguousarray(np.asarray(inputs["x"], dtype=np.float32))
    w_qkv = np.asarray(inputs["w_qkv"], dtype=np.float32)
    b_qkv = np.asarray(inputs["b_qkv"], dtype=np.float32)
    w_out = np.asarray(inputs["w_out"], dtype=np.float32)
    b_out = np.asarray(inputs["b_out"], dtype=np.float32)
    gn_scale = np.asarray(inputs["gn_scale"], dtype=np.float32)
    gn_bias = np.asarray(inputs["gn_bias"], dtype=np.float32)

    fmask = np.zeros((CT, P, G), dtype=np.float32)
    for t in range(CT):
        for p in range(P):
            fmask[t, p, (t * P + p) // GSZ] = 1.0
    bmask = np.ascontiguousarray(fmask.transpose(0, 2, 1))
    # t0 feeds (mean, E[x^2]) per channel; t1 feeds raw (sum, sqsum)
    fmask[0] *= 1.0 / GSZ
    fmask[1] *= 1.0 / (GSZ * N)

    w_q = w_qkv[0:C]
    w_k = w_qkv[C:2 * C]
    w_v = w_qkv[2 * C:3 * C]
    w_ov = w_out @ w_v                                # folded output projection

    bias5 = np.stack([
        b_qkv[0:C],                            # b_q
        b_qkv[C:2 * C],                        # b_k
        b_out + w_out @ b_qkv[2 * C:3 * C],    # b_o (with folded b_v)
        gn_scale,
        gn_bias,
    ])                                          # [5, C]
    common = {
        "wq_p": _pack_w(w_q),
        "wk_p": _pack_w(w_k),
        "wov_p": _pack_w(w_ov),
        "bias5": np.ascontiguousarray(
            bias5.reshape(5, CT, P).transpose(2, 0, 1)),
        "fmask": fmask,
        "bmask": bmask,
    }
    return [dict(common,
                 x=np.ascontiguousarray(x[b].reshape(C, N)),
                 xb=np.ascontiguousarray(x[b].reshape(C, N).astype(NP_BF16)))
            for b in range(B)]


def run(inputs, trace=False):
    nc = _get_program()
    in_maps = make_in_maps(inputs)
    res = bass_utils.run_bass_kernel_spmd(nc, in_maps, core_ids=list(range(B)),
                                          trace=trace)
    out = np.stack([res.results[b]["out"] for b in range(B)])
    return out.reshape(B, C, HH, WW), res


def kernel(**inputs):
    out, _ = run(inputs, trace=False)
    return out


# revision 49
# speedup vs baseline: 1.0046x; 1.0046x over previous
"""AttentionBlock (GroupNorm + single-head NxN attention + residual) on 8 TRN2 cores.

Data-parallel: batch dim (B=8) sharded 1 image per NeuronCore. Each core runs
the full block for its image:

  x (C=256, N=4096) -> GroupNorm(8 groups) -> q,k = Wq,Wk @ xn (fp8)
  u = (W_out @ W_v) @ xn (fp8, output projection folded into V)
  s = k^T q (fp8 DoubleRow matmul, contraction C=256 in one pass)
  e = exp(s/16 - 2) (scalar engine, fp8 out; the -2 shift cancels in softmax)
  attn_u = u @ e, den = ones @ e (fp8 DoubleRow)
  out = attn_u * (1/den) + b_out' + x

All heavy matmuls run as fp8e4m3 with MatmulPerfMode.DoubleRow (K=256 per
matmul). The softmax is unnormalized; 1/den commutes through the (folded)
output projection and is applied once at the end. fp8 noise only touches the
attention branch, which is small versus the fp32 residual, keeping max-rel
error ~1e-3.
"""

import sys

if "/opt/trn_rl_repo" not in sys.path:
    sys.path.insert(0, "/opt/trn_rl_repo")

import numpy as np
import ml_dtypes

import concourse.bass as bass
import concourse.bacc as bacc
import concourse.tile as tile
import concourse.mybir as mybir
from concourse import bass_utils

# Problem dims (hardcoded per spec)
B, C, HH, WW = 8, 256, 64, 64
N = HH * WW            # 4096
G = 8                  # groupnorm groups
GSZ = C // G           # 32 channels/group
EPS = 1e-5
P = 128                # SBUF partitions
CT = C // P            # 2 channel tiles (also the DoubleRow K-tile count)
NCH = 512              # query-chunk width (free dim per matmul)
NNCH = N // NCH        # 8
MT = N // P            # 32 key tiles of 128
JT = MT // 2           # 16 key supertiles of 256 (DoubleRow)
SCALE = 1.0 / np.sqrt(C)
SHIFT = 4.0            # exp(s*SCALE - SHIFT); cancels in softmax, keeps e well
                       # inside fp8e4m3 range (max observed s*SCALE is ~8 ->
                       # e^4 = 55 << 240; overflow headroom up to s*SCALE=9.4)
INV_CNT = 1.0 / (GSZ * N)

F32 = mybir.dt.float32
F32R = mybir.dt.float32r
F8 = mybir.dt.float8e4
BF16 = mybir.dt.bfloat16
DR = mybir.MatmulPerfMode.DoubleRow
NP_F8 = ml_dtypes.float8_e4m3
NP_BF16 = ml_dtypes.bfloat16


def _emit(tc, d, out_d):
    from contextlib import ExitStack

    nc = tc.nc
    AF = mybir.ActivationFunctionType
    OP = mybir.AluOpType
    AX = mybir.AxisListType.X
    ts, ds = bass.ts, bass.ds

    with ExitStack() as ctx:
        const = ctx.enter_context(tc.tile_pool(name="const", bufs=1))
        big = ctx.enter_context(tc.tile_pool(name="big", bufs=1))
        work = ctx.enter_context(tc.tile_pool(name="work", bufs=4))
        small = ctx.enter_context(tc.tile_pool(name="small", bufs=4))
        outp = ctx.enter_context(tc.tile_pool(name="outp", bufs=3))

        # ---------------- load x (bf16 copy first; fp32 in background) ------
        # xb (bf16) feeds GroupNorm stats + xn and lands fast; the fp32 x is
        # only needed for the residual add during finalize (~40us later), so
        # its DMA is issued after everything else on the rings.
        NC4 = 4                      # chunks per channel-tile
        CW = N // NC4                # 1024 columns per chunk
        x_d = d["x"]
        xb_d = d["xb"]
        x_sb = big.tile([P, CT, N], F32, name="x_sb")
        xb_sb = big.tile([P, CT, N], BF16, name="xb_sb")
        dma_engs = (nc.sync, nc.gpsimd, nc.scalar)
        for t in range(CT):
            for c in range(NC4):
                eng = dma_engs[(t * NC4 + c) % len(dma_engs)]
                eng.dma_start(out=xb_sb[:, t, ds(c * CW, CW)],
                              in_=xb_d[ts(t, P), ds(c * CW, CW)])

        fm_sb = const.tile([P, CT, G], F32, name="fm_sb")
        bm_sb = const.tile([G, CT, P], F32, name="bm_sb")
        for t in range(CT):
            nc.sync.dma_start(out=fm_sb[:, t, :], in_=d["fmask"][t])
            nc.sync.dma_start(out=bm_sb[:, t, :], in_=d["bmask"][t])

        # bias5 rows: 0=b_q 1=b_k 2=b_o 3=gn_w 4=gn_b; SBUF [P, 5, CT]
        b5_sb = const.tile([P, 5, CT], F32, name="b5_sb")
        nc.sync.dma_start(out=b5_sb, in_=d["bias5"])
        bq_sb = b5_sb[:, 0, :]
        bk_sb = b5_sb[:, 1, :]
        bo_sb = b5_sb[:, 2, :]
        gw_sb = b5_sb[:, 3, :]
        gb_sb = b5_sb[:, 4, :]

        wq_sb = const.tile([P, CT, C], F8, name="wq_sb")
        wk_sb = const.tile([P, CT, C], F8, name="wk_sb")
        wov_sb = const.tile([P, CT, C], F8, name="wov_sb")
        nc.sync.dma_start(out=wq_sb, in_=d["wq_p"])
        nc.sync.dma_start(out=wk_sb, in_=d["wk_p"])
        nc.sync.dma_start(out=wov_sb, in_=d["wov_p"])

        ones_sb = const.tile([P, CT, P], F8, name="ones_sb")
        nc.vector.memset(ones_sb, 1.0)
        zero_bf = const.tile([P, 1], BF16, name="zero_bf")
        nc.vector.memset(zero_bf, 0.0)
        nshift_sb = const.tile([P, 1], F32, name="nshift_sb")
        nc.vector.memset(nshift_sb, -SHIFT)
        eps_sb = const.tile([G, 1], F32, name="eps_sb")
        nc.vector.memset(eps_sb, EPS)

        # ------- GroupNorm stats: bn_stats for t0 (vector) + sums/squares
        # for t1 (vector+scalar). fmask is pre-scaled host-side so the group
        # matmul directly yields (mean_g, E[x^2]_g).
        NC8 = 8                      # bn_stats chunks (<=512 wide) per ct
        SW = N // NC8                # 512
        st6 = small.tile([P, NC8, 6], F32, name="st6")
        cstat = small.tile([P, 2], F32, name="cstat")   # t0 (mean, var)
        mex = small.tile([P, 2], F32, name="mex")       # t0 (mean, E[x^2])
        stat1 = small.tile([P, NC4, 2], F32, name="stat1")  # t1 (sum, sqsum)
        sqscr = small.tile([P, CW], F32, name="sqscr", bufs=2)
        zero_f = const.tile([P, 1], F32, name="zero_f")
        nc.vector.memset(zero_f, 0.0)
        ab = small.tile([P, CT, 2], F32, name="ab")  # per-channel scale, bias
        with tc.tile_pool(name="psI", bufs=1, space="PSUM") as psI:
            for c in range(NC8):
                nc.vector.bn_stats(out=st6[:, c, :],
                                   in_=xb_sb[:, 0, ds(c * SW, SW)])
                if c % 2 == 0:
                    # PE warm-up: bf16 matmul on the freshly-landed chunk
                    warm = psI.tile([1, NCH], F32, tag="warm", name="warm")
                    nc.tensor.matmul(warm, lhsT=zero_bf,
                                     rhs=xb_sb[:, 0, ds(c * SW, NCH)],
                                     start=True, stop=True)
            for c in range(NC4):
                csl = ds(c * CW, CW)
                nc.scalar.activation(out=sqscr, in_=xb_sb[:, 1, csl],
                                     func=AF.Square, bias=zero_f,
                                     accum_out=stat1[:, c, 1:2])
                nc.vector.reduce_sum(out=stat1[:, c, 0:1],
                                     in_=xb_sb[:, 1, csl], axis=AX)
            nc.vector.bn_aggr(out=cstat, in_=st6)
            # mex = (mean, E[x^2]) for the group aggregation
            nc.vector.tensor_copy(out=mex[:, 0:1], in_=cstat[:, 0:1])
            nc.vector.tensor_mul(out=mex[:, 1:2], in0=cstat[:, 0:1],
                                 in1=cstat[:, 0:1])
            nc.vector.tensor_add(out=mex[:, 1:2], in0=mex[:, 1:2],
                                 in1=cstat[:, 1:2])

            gps = psI.tile([G, 2], F32, tag="gps", name="gps")
            nc.tensor.matmul(gps, lhsT=fm_sb[:, 0, :], rhs=mex,
                             start=True, stop=False)
            for c in range(NC4):
                nc.tensor.matmul(gps, lhsT=fm_sb[:, 1, :], rhs=stat1[:, c, :],
                                 start=False, stop=(c == NC4 - 1))
            grp = small.tile([G, 2], F32, name="grp")    # [mean, rstd]
            gtmp = small.tile([G, 3], F32, name="gtmp")
            nc.vector.tensor_copy(out=grp[:, 0:1], in_=gps[:, 0:1])
            nc.vector.tensor_mul(out=gtmp[:, 1:2], in0=grp[:, 0:1], in1=grp[:, 0:1])
            nc.vector.tensor_sub(out=gtmp[:, 2:3], in0=gps[:, 1:2], in1=gtmp[:, 1:2])
            nc.scalar.activation(out=gtmp[:, 2:3], in_=gtmp[:, 2:3], func=AF.Sqrt,
                                 bias=eps_sb)
            nc.vector.reciprocal(out=grp[:, 1:2], in_=gtmp[:, 2:3])

            cps = psI.tile([P, CT, 2], F32, tag="cps", name="cps")
            for t in range(CT):
                nc.tensor.matmul(cps[:, t, :], lhsT=bm_sb[:, t, :], rhs=grp,
                                 start=True, stop=True)
            nc.vector.tensor_mul(out=ab[:, :, 0], in0=cps[:, :, 1],
                                 in1=gw_sb)
            nc.vector.tensor_mul(out=ab[:, :, 1], in0=cps[:, :, 0],
                                 in1=ab[:, :, 0])
            nc.vector.tensor_sub(out=ab[:, :, 1], in0=gb_sb,
                                 in1=ab[:, :, 1])

        # -------- xn = a*x + b in fp8; low columns on vector (gate the
        # prestage), high columns on gpsimd concurrently -------------------
        xn_sb = big.tile([P, CT, N], F8, name="xn_sb")
        XLO = 2048
        for t in (1, 0):
            nc.vector.tensor_scalar(out=xn_sb[:, t, 0:XLO],
                                    in0=xb_sb[:, t, 0:XLO],
                                    scalar1=ab[:, t, 0:1], scalar2=ab[:, t, 1:2],
                                    op0=OP.mult, op1=OP.add)
        for t in (1, 0):
            nc.gpsimd.tensor_scalar(out=xn_sb[:, t, XLO:N],
                                    in0=xb_sb[:, t, XLO:N],
                                    scalar1=ab[:, t, 0:1], scalar2=ab[:, t, 1:2],
                                    op0=OP.mult, op1=OP.add)

        # fp32 x for the residual: issued now, needed from the first finalize
        for t in range(CT):
            for c in range(NC4):
                eng = (nc.sync, nc.gpsimd)[(t * NC4 + c) % 2]
                eng.dma_start(out=x_sb[:, t, ds(c * CW, CW)],
                              in_=x_d[ts(t, P), ds(c * CW, CW)])

        q_sb = big.tile([P, CT, N], F8, name="q_sb")   # (c, n)
        k_sb = big.tile([P, CT, N], F8, name="k_sb")
        u_sb = big.tile([P, JT, CT, C], F8, name="u_sb")  # uT packed (m, c)

        def emit_q(pool, nch):
            """q chunk pair (both tq) for one nch; copies on vector."""
            nsl = ds(nch * NCH, NCH)
            pr = pool.tile([P, CT, NCH], F32, tag="s", name="psq")
            for tq in range(CT):
                nc.tensor.matmul(pr[:, tq, :], lhsT=wq_sb[:, :, ts(tq, P)],
                                 rhs=xn_sb[:, :, nsl],
                                 start=True, stop=True, perf_mode=DR)
            for tq in range(CT):
                nc.vector.tensor_scalar_add(out=q_sb[:, tq, nsl],
                                            in0=pr[:, tq, :],
                                            scalar1=bq_sb[:, tq:tq + 1])

        def emit_k(pool, c, eng_idx):
            """k chunk pair (both tq) for nch_k c; copies on scalar/vector."""
            nsl = ds(c * NCH, NCH)
            pr = pool.tile([P, CT, NCH], F32, tag="s", name="psk")
            for tq in range(CT):
                nc.tensor.matmul(pr[:, tq, :], lhsT=wk_sb[:, :, ts(tq, P)],
                                 rhs=xn_sb[:, :, nsl],
                                 start=True, stop=True, perf_mode=DR)
            for tq in range(CT):
                if eng_idx == 0:
                    nc.scalar.activation(out=k_sb[:, tq, nsl], in_=pr[:, tq, :],
                                         func=AF.Identity,
                                         bias=bk_sb[:, tq:tq + 1])
                else:
                    nc.vector.tensor_scalar_add(out=k_sb[:, tq, nsl],
                                                in0=pr[:, tq, :],
                                                scalar1=bk_sb[:, tq:tq + 1])

        def emit_u(pool, g, eng_idx):
            """u supertiles 2g, 2g+1 (mt 4g..4g+3); one whole-pair copy."""
            pr = pool.tile([P, CT, NCH], F32, tag="s", name="psu")
            for h in range(4):
                mt = 4 * g + h
                nc.tensor.matmul(pr[:, h // 2, ds((h % 2) * C, C)],
                                 lhsT=xn_sb[:, :, ts(mt, P)],
                                 rhs=wov_sb, start=True, stop=True, perf_mode=DR)
            dst = u_sb[:, 2 * g:2 * g + 2, :, :]
            if eng_idx == 0:
                nc.scalar.copy(out=dst, in_=pr)
            else:
                nc.vector.tensor_copy(out=dst, in_=pr)

        # -- pre-stage q0,q1, k c0-3, u g0-5; the rest rides the stream.
        # u g4/g5 go last: they are the only consumers of the gpsimd-computed
        # high xn columns.
        with tc.tile_pool(name="psQ", bufs=4, space="PSUM") as psQ:
            emit_q(psQ, 0)
            emit_k(psQ, 0, 0)
            emit_u(psQ, 0, 0)
            emit_q(psQ, 1)
            for c in range(1, 4):
                emit_k(psQ, c, 0)
            for g in range(1, 6):
                emit_u(psQ, g, 1 if g % 2 else 0)
            # Exp act-table preload after the last Identity/Copy activation
            # on scalar, so it isn't evicted before the exp stream begins
            nc.scalar.activation(out=gtmp[:, 0:1], in_=gtmp[:, 1:2],
                                 func=AF.Exp, bias=eps_sb)

        # ---------------- main PSUM pools (after psQ released) ---------------
        # 4 banks scores pairs + 2 attn + 1 den + 1 insertion = 8
        psS = ctx.enter_context(tc.tile_pool(name="psS", bufs=2, space="PSUM"))
        psA = ctx.enter_context(tc.tile_pool(name="psA", bufs=1, space="PSUM"))
        psD = ctx.enter_context(tc.tile_pool(name="psD", bufs=1, space="PSUM"))
        psU = ctx.enter_context(tc.tile_pool(name="psU", bufs=1, space="PSUM"))

        def ins_half(kind, a, b=0):
            """Single-bank in-stream production: one k/q chunk half or one u
            supertile, with its copy on vector."""
            pu = psU.tile([P, NCH], F32, tag="u1", name="pu")
            if kind == "u":
                for i in range(2):
                    nc.tensor.matmul(pu[:, ds(i * C, C)],
                                     lhsT=xn_sb[:, :, ts(2 * a + i, P)],
                                     rhs=wov_sb, start=True, stop=True,
                                     perf_mode=DR)
                nc.vector.tensor_copy(out=u_sb[:, a, :, :], in_=pu)
                return
            w_sb, b_sb, o_sb = ((wq_sb, bq_sb, q_sb) if kind == "q"
                                else (wk_sb, bk_sb, k_sb))
            nsl = ds(a * NCH, NCH)
            nc.tensor.matmul(pu, lhsT=w_sb[:, :, ts(b, P)],
                             rhs=xn_sb[:, :, nsl],
                             start=True, stop=True, perf_mode=DR)
            nc.vector.tensor_scalar_add(out=o_sb[:, b, nsl], in0=pu,
                                        scalar1=b_sb[:, b:b + 1])

        # deferred work for the nch0 sweep, deadline-ordered (k c needed at
        # pair j=2c; u supertile j' needed at attn j'); j0/j1 are reserved
        # for the ramp-up scores on psU
        INS0 = {2: ("k", 4, 0), 3: ("k", 4, 1), 4: ("k", 5, 0), 5: ("k", 5, 1),
                6: ("k", 6, 0), 7: ("k", 6, 1), 8: ("u", 12), 9: ("u", 13),
                10: ("k", 7, 0), 11: ("k", 7, 1), 12: ("u", 14), 13: ("u", 15)}

        # ---------------- attention + fused output projection ----------------
        # Software pipeline over all (nch, j): scores for step idx+1 are
        # emitted before attn/den of step idx so the PE keeps the scalar
        # engine's exp stream fed.
        steps = [(nch, j) for nch in range(NNCH) for j in range(JT)]

        def emit_scores(nch, j):
            nsl = ds(nch * NCH, NCH)
            pr = psS.tile([P, CT, NCH], F32, tag="s", name="pss")
            for i in range(2):
                mt = 2 * j + i
                nc.tensor.matmul(pr[:, i, :], lhsT=k_sb[:, :, ts(mt, P)],
                                 rhs=q_sb[:, :, nsl],
                                 start=True, stop=True, perf_mode=DR)
            return pr

        attn = None
        den = None
        pr_cur = None
        for idx, (nch, j) in enumerate(steps):
            nsl = ds(nch * NCH, NCH)
            if j == 0:
                attn = psA.tile([P, CT, NCH], F32, tag="attn", name="attn")
                den = psD.tile([P, NCH], F32, tag="den", name="den")
            e = work.tile([P, CT, NCH], F8, tag="e", name="e", bufs=4)
            if idx < 2:
                # ramp-up: single-bank scores on psU + single-tile exps, so
                # the exp stream starts before the prestage copies drain out
                # of the psQ banks that psS is about to reuse
                for i in range(2):
                    pu = psU.tile([P, NCH], F32, tag="u1", name="pss1")
                    nc.tensor.matmul(pu, lhsT=k_sb[:, :, ts(2 * j + i, P)],
                                     rhs=q_sb[:, :, nsl],
                                     start=True, stop=True, perf_mode=DR)
                    nc.scalar.activation(out=e[:, i, :], in_=pu, func=AF.Exp,
                                         bias=nshift_sb, scale=SCALE)
            else:
                nc.scalar.activation(out=e, in_=pr_cur, func=AF.Exp,
                                     bias=nshift_sb, scale=SCALE)
            if 2 <= idx + 1 < len(steps):
                pr_cur = emit_scores(*steps[idx + 1])
            if nch == 0:
                if j in INS0:
                    ins_half(*INS0[j])
            elif nch + 1 < NNCH and j in (9, 11):
                ins_half("q", nch + 1, 0 if j == 9 else 1)
            for co in range(CT):
                nc.tensor.matmul(attn[:, co, :],
                                 lhsT=u_sb[:, j, :, ts(co, P)],
                                 rhs=e,
                                 start=(j == 0), stop=(j == JT - 1),
                                 perf_mode=DR)
            nc.tensor.matmul(den, lhsT=ones_sb, rhs=e,
                             start=(j == 0), stop=(j == JT - 1),
                             perf_mode=DR)
            if j != JT - 1:
                continue

            # -------- finalize this nch --------
            # pull attn out of PSUM first so the next nch's attn matmuls can
            # restart the accumulation group without waiting on the chain
            att_sb = outp.tile([P, CT, NCH], F32, tag="att_sb", name="att_sb",
                               bufs=2)
            # two bank-sized copies so each attn PSUM bank frees independently;
            # for the last nch the scalar engine is done with exps, so it
            # takes them and shortens the serial tail on vector
            for co in range(CT):
                if nch == NNCH - 1:
                    nc.scalar.copy(out=att_sb[:, co, :], in_=attn[:, co, :])
                else:
                    nc.vector.tensor_copy(out=att_sb[:, co, :],
                                          in_=attn[:, co, :])
            rden = outp.tile([P, NCH], F32, tag="rden", name="rden", bufs=2)
            rscr = outp.tile([P, NCH], F32, tag="rscr", name="rscr", bufs=2)
            nc.vector.reciprocal_approx_accurate(out=rden, in_=den, scratch=rscr)
            for co in range(CT):
                f = outp.tile([P, NCH], F32, tag="fout", name="f", bufs=3)
                nc.vector.tensor_tensor(out=f, in0=att_sb[:, co, :], in1=rden,
                                        op=OP.mult)
                nc.vector.scalar_tensor_tensor(out=f, in0=f,
                                               scalar=bo_sb[:, co:co + 1],
                                               in1=x_sb[:, co, nsl],
                                               op0=OP.add, op1=OP.add)
                nc.sync.dma_start(out=out_d[ts(co, P), nsl], in_=f)


def build_program():
    nc = bacc.Bacc("TRN2", target_bir_lowering=False, debug=False, num_devices=B)
    d = {}

    def din(name, shape, dt_=F32):
        d[name] = nc.dram_tensor(name, list(shape), dt_, kind="ExternalInput").ap()

    din("x", (C, N))
    din("xb", (C, N), BF16)
    din("wq_p", (P, CT, C), F8)
    din("wk_p", (P, CT, C), F8)
    din("wov_p", (P, CT, C), F8)
    din("bias5", (P, 5, CT))
    din("fmask", (CT, P, G))
    din("bmask", (CT, G, P))
    out_d = nc.dram_tensor("out", [C, N], F32, kind="ExternalOutput").ap()

    with tile.TileContext(nc) as tc:
        _emit(tc, d, out_d)
    nc.compile()
    return nc


_PROG = None


def _get_program():
    global _PROG
    if _PROG is None:
        _PROG = build_program()
    return _PROG


def _pack_w(w):
    """[c_out, c_in] fp32 -> [p, r, c_out] fp8 with c_in = r*128 + p."""
    wt = np.ascontiguousarray(w.T)                   # [c_in, c_out]
    return np.ascontiguousarray(
        wt.reshape(CT, P, C).transpose(1, 0, 2)).astype(NP_F8)


def make_in_maps(inputs):
    x = np.ascontiguousarray(np.asarray(inputs["x"], dtype=np.float32))
    w_qkv = np.asarray(inputs["w_qkv"], dtype=np.float32)
    b_qkv = np.asarray(inputs["b_qkv"], dtype=np.float32)
    w_out = np.asarray(inputs["w_out"], dtype=np.float32)
    b_out = np.asarray(inputs["b_out"], dtype=np.float32)
    gn_scale = np.asarray(inputs["gn_scale"], dtype=np.float32)
    gn_bias = np.asarray(inputs["gn_bias"], dtype=np.float32)

    fmask = np.zeros((CT, P, G), dtype=np.float32)
    for t in range(CT):
        for p in range(P):
            fmask[t, p, (t * P + p) // GSZ] = 1.0
    bmask = np.ascontiguousarray(fmask.transpose(0, 2, 1))
    # t0 feeds (mean, E[x^2]) per channel; t1 feeds raw (sum, sqsum)
    fmask[0] *= 1.0 / GSZ
    fmask[1] *= 1.0 / (GSZ * N)

    w_q = w_qkv[0:C]
    w_k = w_qkv[C:2 * C]
    w_v = w_qkv[2 * C:3 * C]
    w_ov = w_out @ w_v                                # folded output projection

    bias5 = np.stack([
        b_qkv[0:C],                            # b_q
        b_qkv[C:2 * C],                        # b_k
        b_out + w_out @ b_qkv[2 * C:3 * C],    # b_o (with folded b_v)
        gn_scale,
        gn_bias,
    ])                                          # [5, C]
    common = {
        "wq_p": _pack_w(w_q),
        "wk_p": _pack_w(w_k),
        "wov_p": _pack_w(w_ov),
        "bias5": np.ascontiguousarray(
            bias5.reshape(5, CT, P).transpose(2, 0, 1)),
        "fmask": fmask,
        "bmask": bmask,
    }
    return [dict(common,
                 x=np.ascontiguousarray(x[b].reshape(C, N)),
                 xb=np.ascontiguousarray(x[b].reshape(C, N).astype(NP_BF16)))
            for b in range(B)]


def run(inputs, trace=False):
    nc = _get_program()
    in_maps = make_in_maps(inputs)
    res = bass_utils.run_bass_kernel_spmd(nc, in_maps, core_ids=list(range(B)),
                                          trace=trace)
    out = np.stack([res.results[b]["out"] for b in range(B)])
    return out.reshape(B, C, HH, WW), res


def kernel(**inputs):
    out, _ = run(inputs, trace=False)
    return out


# revision 51
# speedup vs baseline: 1.2808x; 1.2749x over previous
"""AttentionBlock (GroupNorm + single-head NxN attention + residual) on 8 TRN2 cores.

Data-parallel: batch dim (B=8) sharded 1 image per NeuronCore. Each core runs
the full block for its image:

  x (C=256, N=4096) -> GroupNorm(8 groups) -> q,k = Wq,Wk @ xn (fp8)
  u = (W_out @ W_v) @ xn (fp8, output projection folded into V)
  s = k^T q (fp8 DoubleRow matmul, contraction C=256 in one pass)
  e = exp(s/16 - 2) (scalar engine, fp8 out; the -2 shift cancels in softmax)
  attn_u = u @ e, den = ones @ e (fp8 DoubleRow)
  out = attn_u * (1/den) + b_out' + x

All heavy matmuls run as fp8e4m3 with MatmulPerfMode.DoubleRow (K=256 per
matmul). The softmax is unnormalized; 1/den commutes through the (folded)
output projection and is applied once at the end. fp8 noise only touches the
attention branch, which is small versus the fp32 residual, keeping max-rel
error ~1e-3.
"""

import sys

if "/opt/trn_rl_repo" not in sys.path:
    sys.path.insert(0, "/opt/trn_rl_repo")

import numpy as np
import ml_dtypes

import concourse.bass as bass
import concourse.bacc as bacc
import concourse.tile as tile
import concourse.mybir as mybir
from concourse import bass_utils

# Problem dims (hardcoded per spec)
B, C, HH, WW = 8, 256, 64, 64
N = HH * WW            # 4096
G = 8                  # groupnorm groups
GSZ = C // G           # 32 channels/group
EPS = 1e-5
P = 128                # SBUF partitions
CT = C // P            # 2 channel tiles (also the DoubleRow K-tile count)
NCH = 512              # query-chunk width (free dim per matmul)
NNCH = N // NCH        # 8
MT = N // P            # 32 key tiles of 128
JT = MT // 2           # 16 key supertiles of 256 (DoubleRow)
SCALE = 1.0 / np.sqrt(C)
SHIFT = 4.0            # exp(s*SCALE - SHIFT); cancels in softmax, keeps e well
                       # inside fp8e4m3 range (max observed s*SCALE is ~8 ->
                       # e^4 = 55 << 240; overflow headroom up to s*SCALE=9.4)
INV_CNT = 1.0 / (GSZ * N)

F32 = mybir.dt.float32
F32R = mybir.dt.float32r
F8 = mybir.dt.float8e4
BF16 = mybir.dt.bfloat16
DR = mybir.MatmulPerfMode.DoubleRow
NP_F8 = ml_dtypes.float8_e4m3
NP_BF16 = ml_dtypes.bfloat16


def _emit(tc, d, out_d):
    from contextlib import ExitStack

    nc = tc.nc
    AF = mybir.ActivationFunctionType
    OP = mybir.AluOpType
    AX = mybir.AxisListType.X
    ts, ds = bass.ts, bass.ds

    with ExitStack() as ctx:
        const = ctx.enter_context(tc.tile_pool(name="const", bufs=1))
        big = ctx.enter_context(tc.tile_pool(name="big", bufs=1))
        work = ctx.enter_context(tc.tile_pool(name="work", bufs=4))
        small = ctx.enter_context(tc.tile_pool(name="small", bufs=4))
        outp = ctx.enter_context(tc.tile_pool(name="outp", bufs=3))

        # ---------------- load x (bf16 copy first; fp32 in background) ------
        # xb (bf16) feeds GroupNorm stats + xn and lands fast; the fp32 x is
        # only needed for the residual add during finalize (~40us later), so
        # its DMA is issued after everything else on the rings.
        NC4 = 4                      # chunks per channel-tile
        CW = N // NC4                # 1024 columns per chunk
        x_d = d["x"]
        xb_d = d["xb"]
        x_sb = big.tile([P, CT, N], F32, name="x_sb")
        xb_sb = big.tile([P, CT, N], BF16, name="xb_sb")
        dma_engs = (nc.sync, nc.gpsimd, nc.scalar)
        for t in range(CT):
            for c in range(NC4):
                eng = dma_engs[(t * NC4 + c) % len(dma_engs)]
                eng.dma_start(out=xb_sb[:, t, ds(c * CW, CW)],
                              in_=xb_d[ts(t, P), ds(c * CW, CW)])

        fm_sb = const.tile([P, CT, G], F32, name="fm_sb")
        bm_sb = const.tile([G, CT, P], F32, name="bm_sb")
        for t in range(CT):
            nc.sync.dma_start(out=fm_sb[:, t, :], in_=d["fmask"][t])
            nc.sync.dma_start(out=bm_sb[:, t, :], in_=d["bmask"][t])

        # bias5 rows: 0=b_q 1=b_k 2=b_o 3=gn_w 4=gn_b; SBUF [P, 5, CT]
        b5_sb = const.tile([P, 5, CT], F32, name="b5_sb")
        nc.sync.dma_start(out=b5_sb, in_=d["bias5"])
        bq_sb = b5_sb[:, 0, :]
        bk_sb = b5_sb[:, 1, :]
        bo_sb = b5_sb[:, 2, :]
        gw_sb = b5_sb[:, 3, :]
        gb_sb = b5_sb[:, 4, :]

        wq_sb = const.tile([P, CT, C], F8, name="wq_sb")
        wk_sb = const.tile([P, CT, C], F8, name="wk_sb")
        wov_sb = const.tile([P, CT, C], F8, name="wov_sb")
        nc.sync.dma_start(out=wq_sb, in_=d["wq_p"])
        nc.sync.dma_start(out=wk_sb, in_=d["wk_p"])
        nc.sync.dma_start(out=wov_sb, in_=d["wov_p"])

        ones_sb = const.tile([P, CT, P], F8, name="ones_sb")
        nc.vector.memset(ones_sb, 1.0)
        zero_bf = const.tile([P, 1], BF16, name="zero_bf")
        nc.vector.memset(zero_bf, 0.0)
        nshift_sb = const.tile([P, 1], F32, name="nshift_sb")
        nc.vector.memset(nshift_sb, -SHIFT)
        eps_sb = const.tile([G, 1], F32, name="eps_sb")
        nc.vector.memset(eps_sb, EPS)

        # ------- GroupNorm stats: bn_stats for t0 (vector) + sums/squares
        # for t1 (vector+scalar). fmask is pre-scaled host-side so the group
        # matmul directly yields (mean_g, E[x^2]_g).
        NC8 = 8                      # bn_stats chunks (<=512 wide) per ct
        SW = N // NC8                # 512
        st6 = small.tile([P, NC8, 6], F32, name="st6")
        cstat = small.tile([P, 2], F32, name="cstat")   # t0 (mean, var)
        mex = small.tile([P, 2], F32, name="mex")       # t0 (mean, E[x^2])
        stat1 = small.tile([P, NC4, 2], F32, name="stat1")  # t1 (sum, sqsum)
        sqscr = small.tile([P, CW], F32, name="sqscr", bufs=2)
        zero_f = const.tile([P, 1], F32, name="zero_f")
        nc.vector.memset(zero_f, 0.0)
        ab = small.tile([P, CT, 2], F32, name="ab")  # per-channel scale, bias
        with tc.tile_pool(name="psI", bufs=1, space="PSUM") as psI:
            for c in range(NC8):
                nc.vector.bn_stats(out=st6[:, c, :],
                                   in_=xb_sb[:, 0, ds(c * SW, SW)])
                if c % 2 == 0:
                    # PE warm-up: bf16 matmul on the freshly-landed chunk
                    warm = psI.tile([1, NCH], F32, tag="warm", name="warm")
                    nc.tensor.matmul(warm, lhsT=zero_bf,
                                     rhs=xb_sb[:, 0, ds(c * SW, NCH)],
                                     start=True, stop=True)
            for c in range(NC4):
                csl = ds(c * CW, CW)
                nc.scalar.activation(out=sqscr, in_=xb_sb[:, 1, csl],
                                     func=AF.Square, bias=zero_f,
                                     accum_out=stat1[:, c, 1:2])
                nc.vector.reduce_sum(out=stat1[:, c, 0:1],
                                     in_=xb_sb[:, 1, csl], axis=AX)
            nc.vector.bn_aggr(out=cstat, in_=st6)
            # mex = (mean, E[x^2]) for the group aggregation
            nc.vector.tensor_copy(out=mex[:, 0:1], in_=cstat[:, 0:1])
            nc.vector.tensor_mul(out=mex[:, 1:2], in0=cstat[:, 0:1],
                                 in1=cstat[:, 0:1])
            nc.vector.tensor_add(out=mex[:, 1:2], in0=mex[:, 1:2],
                                 in1=cstat[:, 1:2])

            gps = psI.tile([G, 2], F32, tag="gps", name="gps")
            nc.tensor.matmul(gps, lhsT=fm_sb[:, 0, :], rhs=mex,
                             start=True, stop=False)
            for c in range(NC4):
                nc.tensor.matmul(gps, lhsT=fm_sb[:, 1, :], rhs=stat1[:, c, :],
                                 start=False, stop=(c == NC4 - 1))
            grp = small.tile([G, 2], F32, name="grp")    # [mean, rstd]
            gtmp = small.tile([G, 3], F32, name="gtmp")
            nc.vector.tensor_copy(out=grp[:, 0:1], in_=gps[:, 0:1])
            nc.vector.tensor_mul(out=gtmp[:, 1:2], in0=grp[:, 0:1], in1=grp[:, 0:1])
            nc.vector.tensor_sub(out=gtmp[:, 2:3], in0=gps[:, 1:2], in1=gtmp[:, 1:2])
            nc.scalar.activation(out=gtmp[:, 2:3], in_=gtmp[:, 2:3], func=AF.Sqrt,
                                 bias=eps_sb)
            nc.vector.reciprocal(out=grp[:, 1:2], in_=gtmp[:, 2:3])

            cps = psI.tile([P, CT, 2], F32, tag="cps", name="cps")
            for t in range(CT):
                nc.tensor.matmul(cps[:, t, :], lhsT=bm_sb[:, t, :], rhs=grp,
                                 start=True, stop=True)
            nc.vector.tensor_mul(out=ab[:, :, 0], in0=cps[:, :, 1],
                                 in1=gw_sb)
            nc.vector.tensor_mul(out=ab[:, :, 1], in0=cps[:, :, 0],
                                 in1=ab[:, :, 0])
            nc.vector.tensor_sub(out=ab[:, :, 1], in0=gb_sb,
                                 in1=ab[:, :, 1])

        # -------- xn = a*x + b in fp8; low columns on vector (gate the
        # prestage), high columns on gpsimd concurrently -------------------
        xn_sb = big.tile([P, CT, N], F8, name="xn_sb")
        XLO = 2048
        for t in (1, 0):
            nc.vector.tensor_scalar(out=xn_sb[:, t, 0:XLO],
                                    in0=xb_sb[:, t, 0:XLO],
                                    scalar1=ab[:, t, 0:1], scalar2=ab[:, t, 1:2],
                                    op0=OP.mult, op1=OP.add)
        for t in (1, 0):
            nc.gpsimd.tensor_scalar(out=xn_sb[:, t, XLO:N],
                                    in0=xb_sb[:, t, XLO:N],
                                    scalar1=ab[:, t, 0:1], scalar2=ab[:, t, 1:2],
                                    op0=OP.mult, op1=OP.add)

        # fp32 x for the residual: issued now, needed from the first finalize
        for t in range(CT):
            for c in range(NC4):
                eng = (nc.sync, nc.gpsimd)[(t * NC4 + c) % 2]
                eng.dma_start(out=x_sb[:, t, ds(c * CW, CW)],
                              in_=x_d[ts(t, P), ds(c * CW, CW)])

        q_sb = big.tile([P, CT, N], F8, name="q_sb")   # (c, n)
        k_sb = big.tile([P, CT, N], F8, name="k_sb")
        u_sb = big.tile([P, JT, CT, C], F8, name="u_sb")  # uT packed (m, c)

        def emit_q(pool, nch):
            """q chunk pair (both tq) for one nch; copies on vector."""
            nsl = ds(nch * NCH, NCH)
            pr = pool.tile([P, CT, NCH], F32, tag="s", name="psq")
            for tq in range(CT):
                nc.tensor.matmul(pr[:, tq, :], lhsT=wq_sb[:, :, ts(tq, P)],
                                 rhs=xn_sb[:, :, nsl],
                                 start=True, stop=True, perf_mode=DR)
            for tq in range(CT):
                nc.vector.tensor_scalar_add(out=q_sb[:, tq, nsl],
                                            in0=pr[:, tq, :],
                                            scalar1=bq_sb[:, tq:tq + 1])

        def emit_k(pool, c, eng_idx):
            """k chunk pair (both tq) for nch_k c; copies on scalar/vector."""
            nsl = ds(c * NCH, NCH)
            pr = pool.tile([P, CT, NCH], F32, tag="s", name="psk")
            for tq in range(CT):
                nc.tensor.matmul(pr[:, tq, :], lhsT=wk_sb[:, :, ts(tq, P)],
                                 rhs=xn_sb[:, :, nsl],
                                 start=True, stop=True, perf_mode=DR)
            for tq in range(CT):
                if eng_idx == 0:
                    nc.scalar.activation(out=k_sb[:, tq, nsl], in_=pr[:, tq, :],
                                         func=AF.Identity,
                                         bias=bk_sb[:, tq:tq + 1])
                else:
                    nc.vector.tensor_scalar_add(out=k_sb[:, tq, nsl],
                                                in0=pr[:, tq, :],
                                                scalar1=bk_sb[:, tq:tq + 1])

        def emit_u(pool, g, eng_idx):
            """u supertiles 2g, 2g+1 (mt 4g..4g+3); one whole-pair copy."""
            pr = pool.tile([P, CT, NCH], F32, tag="s", name="psu")
            for h in range(4):
                mt = 4 * g + h
                nc.tensor.matmul(pr[:, h // 2, ds((h % 2) * C, C)],
                                 lhsT=xn_sb[:, :, ts(mt, P)],
                                 rhs=wov_sb, start=True, stop=True, perf_mode=DR)
            dst = u_sb[:, 2 * g:2 * g + 2, :, :]
            if eng_idx == 0:
                nc.scalar.copy(out=dst, in_=pr)
            else:
                nc.vector.tensor_copy(out=dst, in_=pr)

        # -- pre-stage q0,q1, k c0-3, u g0-5; the rest rides the stream.
        # u g4/g5 go last: they are the only consumers of the gpsimd-computed
        # high xn columns.
        with tc.tile_pool(name="psQ", bufs=4, space="PSUM") as psQ:
            emit_q(psQ, 0)
            emit_k(psQ, 0, 0)
            emit_u(psQ, 0, 0)
            emit_q(psQ, 1)
            for c in range(1, 4):
                emit_k(psQ, c, 0)
            for g in range(1, 6):
                emit_u(psQ, g, 1 if g % 2 else 0)
            # Exp act-table preload after the last Identity/Copy activation
            # on scalar, so it isn't evicted before the exp stream begins
            nc.scalar.activation(out=gtmp[:, 0:1], in_=gtmp[:, 1:2],
                                 func=AF.Exp, bias=eps_sb)

        # ---------------- main PSUM pools (after psQ released) ---------------
        # 4 banks scores pairs + 2 attn + 1 den + 1 insertion = 8
        psS = ctx.enter_context(tc.tile_pool(name="psS", bufs=2, space="PSUM"))
        psA = ctx.enter_context(tc.tile_pool(name="psA", bufs=1, space="PSUM"))
        psD = ctx.enter_context(tc.tile_pool(name="psD", bufs=1, space="PSUM"))
        psU = ctx.enter_context(tc.tile_pool(name="psU", bufs=1, space="PSUM"))

        def ins_half(kind, a, b=0):
            """Single-bank in-stream production: one k/q chunk half or one u
            supertile, with its copy on vector."""
            pu = psU.tile([P, NCH], F32, tag="u1", name="pu")
            if kind == "u":
                for i in range(2):
                    nc.tensor.matmul(pu[:, ds(i * C, C)],
                                     lhsT=xn_sb[:, :, ts(2 * a + i, P)],
                                     rhs=wov_sb, start=True, stop=True,
                                     perf_mode=DR)
                nc.vector.tensor_copy(out=u_sb[:, a, :, :], in_=pu)
                return
            w_sb, b_sb, o_sb = ((wq_sb, bq_sb, q_sb) if kind == "q"
                                else (wk_sb, bk_sb, k_sb))
            nsl = ds(a * NCH, NCH)
            nc.tensor.matmul(pu, lhsT=w_sb[:, :, ts(b, P)],
                             rhs=xn_sb[:, :, nsl],
                             start=True, stop=True, perf_mode=DR)
            nc.vector.tensor_scalar_add(out=o_sb[:, b, nsl], in0=pu,
                                        scalar1=b_sb[:, b:b + 1])

        # deferred work for the nch0 sweep, deadline-ordered (k c needed at
        # pair j=2c; u supertile j' needed at attn j'); j0/j1 are reserved
        # for the ramp-up scores on psU
        INS0 = {2: ("k", 4, 0), 3: ("k", 4, 1), 4: ("k", 5, 0), 5: ("k", 5, 1),
                6: ("k", 6, 0), 7: ("k", 6, 1), 8: ("u", 12), 9: ("u", 13),
                10: ("k", 7, 0), 11: ("k", 7, 1), 12: ("u", 14), 13: ("u", 15)}

        # ---------------- attention + fused output projection ----------------
        # Software pipeline over all (nch, j): scores for step idx+1 are
        # emitted before attn/den of step idx so the PE keeps the scalar
        # engine's exp stream fed.
        steps = [(nch, j) for nch in range(NNCH) for j in range(JT)]

        def emit_scores(nch, j):
            nsl = ds(nch * NCH, NCH)
            pr = psS.tile([P, CT, NCH], F32, tag="s", name="pss")
            for i in range(2):
                mt = 2 * j + i
                nc.tensor.matmul(pr[:, i, :], lhsT=k_sb[:, :, ts(mt, P)],
                                 rhs=q_sb[:, :, nsl],
                                 start=True, stop=True, perf_mode=DR)
            return pr

        def emit_consumers(e, attn, den, nch, j):
            for co in range(CT):
                nc.tensor.matmul(attn[:, co, :],
                                 lhsT=u_sb[:, j, :, ts(co, P)],
                                 rhs=e,
                                 start=(j == 0), stop=(j == JT - 1),
                                 perf_mode=DR)
            nc.tensor.matmul(den, lhsT=ones_sb, rhs=e,
                             start=(j == 0), stop=(j == JT - 1),
                             perf_mode=DR)

        def finalize(attn, den, nch):
            # pull attn out of PSUM first so the next nch's attn matmuls can
            # restart the accumulation group without waiting on the chain
            nsl = ds(nch * NCH, NCH)
            att_sb = outp.tile([P, CT, NCH], F32, tag="att_sb", name="att_sb",
                               bufs=2)
            # two bank-sized copies so each attn PSUM bank frees independently;
            # for the last nch the scalar engine is done with exps, so it
            # takes them and shortens the serial tail on vector
            for co in range(CT):
                if nch == NNCH - 1:
                    nc.scalar.copy(out=att_sb[:, co, :], in_=attn[:, co, :])
                else:
                    nc.vector.tensor_copy(out=att_sb[:, co, :],
                                          in_=attn[:, co, :])
            rden = outp.tile([P, NCH], F32, tag="rden", name="rden", bufs=2)
            rscr = outp.tile([P, NCH], F32, tag="rscr", name="rscr", bufs=2)
            nc.vector.reciprocal_approx_accurate(out=rden, in_=den, scratch=rscr)
            for co in range(CT):
                f = outp.tile([P, NCH], F32, tag="fout", name="f", bufs=3)
                nc.vector.tensor_tensor(out=f, in0=att_sb[:, co, :], in1=rden,
                                        op=OP.mult)
                nc.vector.scalar_tensor_tensor(out=f, in0=f,
                                               scalar=bo_sb[:, co:co + 1],
                                               in1=x_sb[:, co, nsl],
                                               op0=OP.add, op1=OP.add)
                nc.sync.dma_start(out=out_d[ts(co, P), nsl], in_=f)

        attn = None
        den = None
        pr_cur = None
        pending = None   # e-consumers ride one step behind the scores
        for idx, (nch, j) in enumerate(steps):
            nsl = ds(nch * NCH, NCH)
            if j == 0:
                attn = psA.tile([P, CT, NCH], F32, tag="attn", name="attn")
                den = psD.tile([P, NCH], F32, tag="den", name="den")
            e = work.tile([P, CT, NCH], F8, tag="e", name="e", bufs=5)
            if idx < 2:
                # ramp-up: single-bank scores on psU + single-tile exps, so
                # the exp stream starts before the prestage copies drain out
                # of the psQ banks that psS is about to reuse
                for i in range(2):
                    pu = psU.tile([P, NCH], F32, tag="u1", name="pss1")
                    nc.tensor.matmul(pu, lhsT=k_sb[:, :, ts(2 * j + i, P)],
                                     rhs=q_sb[:, :, nsl],
                                     start=True, stop=True, perf_mode=DR)
                    nc.scalar.activation(out=e[:, i, :], in_=pu, func=AF.Exp,
                                         bias=nshift_sb, scale=SCALE)
            else:
                nc.scalar.activation(out=e, in_=pr_cur, func=AF.Exp,
                                     bias=nshift_sb, scale=SCALE)
            if 2 <= idx + 1 < len(steps):
                pr_cur = emit_scores(*steps[idx + 1])
            if pending is not None:
                emit_consumers(*pending)
                if pending[4] == JT - 1:
                    finalize(pending[1], pending[2], pending[3])
            pending = (e, attn, den, nch, j)
            if nch == 0:
                if j in INS0:
                    ins_half(*INS0[j])
            elif nch + 1 < NNCH and j in (9, 11):
                ins_half("q", nch + 1, 0 if j == 9 else 1)
        emit_consumers(*pending)
        finalize(pending[1], pending[2], pending[3])


def build_program():
    nc = bacc.Bacc("TRN2", target_bir_lowering=False, debug=False, num_devices=B)
    d = {}

    def din(name, shape, dt_=F32):
        d[name] = nc.dram_tensor(name, list(shape), dt_, kind="ExternalInput").ap()

    din("x", (C, N))
    din("xb", (C, N), BF16)
    din("wq_p", (P, CT, C), F8)
    din("wk_p", (P, CT, C), F8)
    din("wov_p", (P, CT, C), F8)
    din("bias5", (P, 5, CT))
    din("fmask", (CT, P, G))
    din("bmask", (CT, G, P))
    out_d = nc.dram_tensor("out", [C, N], F32, kind="ExternalOutput").ap()

    with tile.TileContext(nc) as tc:
        _emit(tc, d, out_d)
    nc.compile()
    return nc


_PROG = None


def _get_program():
    global _PROG
    if _PROG is None:
        _PROG = build_program()
    return _PROG


def _pack_w(w):
    """[c_out, c_in] fp32 -> [p, r, c_out] fp8 with c_in = r*128 + p."""
    wt = np.ascontiguousarray(w.T)                   # [c_in, c_out]
    return np.ascontiguousarray(
        wt.reshape(CT, P, C).transpose(1, 0, 2)).astype(NP_F8)


def make_in_maps(inputs):
    x = np.ascontiguousarray(np.asarray(inputs["x"], dtype=np.float32))
    w_qkv = np.asarray(inputs["w_qkv"], dtype=np.float32)
    b_qkv = np.asarray(inputs["b_qkv"], dtype=np.float32)
    w_out = np.asarray(inputs["w_out"], dtype=np.float32)
    b_out = np.asarray(inputs["b_out"], dtype=np.float32)
    gn_scale = np.asarray(inputs["gn_scale"], dtype=np.float32)
    gn_bias = np.asarray(inputs["gn_bias"], dtype=np.float32)

    fmask = np.zeros((CT, P, G), dtype=np.float32)
    for t in range(CT):
        for p in range(P):
            fmask[t, p, (t * P + p) // GSZ] = 1.0
    bmask = np.ascontiguousarray(fmask.transpose(0, 2, 1))
    # t0 feeds (mean, E[x^2]) per channel; t1 feeds raw (sum, sqsum)
    fmask[0] *= 1.0 / GSZ
    fmask[1] *= 1.0 / (GSZ * N)

    w_q = w_qkv[0:C]
    w_k = w_qkv[C:2 * C]
    w_v = w_qkv[2 * C:3 * C]
    w_ov = w_out @ w_v                                # folded output projection

    bias5 = np.stack([
        b_qkv[0:C],                            # b_q
        b_qkv[C:2 * C],                        # b_k
        b_out + w_out @ b_qkv[2 * C:3 * C],    # b_o (with folded b_v)
        gn_scale,
        gn_bias,
    ])                                          # [5, C]
    common = {
        "wq_p": _pack_w(w_q),
        "wk_p": _pack_w(w_k),
        "wov_p": _pack_w(w_ov),
        "bias5": np.ascontiguousarray(
            bias5.reshape(5, CT, P).transpose(2, 0, 1)),
        "fmask": fmask,
        "bmask": bmask,
    }
    return [dict(common,
                 x=np.ascontiguousarray(x[b].reshape(C, N)),
                 xb=np.ascontiguousarray(x[b].reshape(C, N).astype(NP_BF16)))
            for b in range(B)]


def run(inputs, trace=False):
    nc = _get_program()
    in_maps = make_in_maps(inputs)
    res = bass_utils.run_bass_kernel_spmd(nc, in_maps, core_ids=list(range(B)),
                                          trace=trace)
    out = np.stack([res.results[b]["out"] for b in range(B)])
    return out.reshape(B, C, HH, WW), res


def kernel(**inputs):
    out, _ = run(inputs, trace=False)
    return out
